# revision 1
# baseline (speedup 1.0000x reference)
"""Self-contained Trainium2 Bass kernel for the GAT layer problem
nn_GATLayer_57062935494774 (V=50000, E=800000, IN=256, OUT=128, alpha=0.2).

kernel(**inputs) takes the full unsharded inputs (x, W, a, edge_index),
distributes across 8 NeuronCores, and returns the full (V, 128) output.

v3 design:
  The per-slot indirect gather (one SWDGE instruction per 128-edge slot,
  ~1.4 us each on the single GpSimd engine) is the hard wall at ~1.13 ms,
  so everything else is arranged to hide underneath it:
  - No phase barrier.  Table-block writes bump a semaphore; each gather
    unit waits only for the table prefix it actually reads (per-partition
    edge lists are sorted by column, so early slots need only an early
    prefix).  Gather units are emitted in ascending watermark order, so
    GpSimd starts ~40 us into the table build and never stalls again.
  - Phase 1: bf16 matmuls produce [Wh | s_dst | s_src] in fp32 PSUM;
    features cast to bf16, the two scores kept as packed fp32 in a
    264-byte table row; rows written with an even/odd interleave so each
    descriptor carries two rows (528 B, no RMW penalty).  The 176 spare
    rows sit at the FRONT of the table as trash rows whose crafted x
    gives s_dst = -1e4 -> exp == 0 exactly (padding needs no mask and
    no watermark).
  - Score path per 4-slot band: u = s_dst_strided + s_src (fast DVE),
    lrelu on DVE, exp on ACT into a persistent phi tile; aggregation is
    one fused scalar_tensor_tensor (acc = G*phi + acc, in place) per
    slot; denominator via one reduce_sum at tile end; elu output via
    3 fast DVE ops + 1 ACT exp.
"""

import numpy as np

P = 128
TW = 132          # table row width in bf16 elems (264 B): 128 feats + 2 fp32
ALPHA = 0.2
NCORES = 8
TPC = 49          # row tiles per core (8*49*128 = 50176 >= 50000)
XB = 8            # phase-1 node blocks per big tile (1024 rows per write)
BAND = 4          # gather slots per emission unit


# ------------------------------------------------------------------ fixes

def _install_legalizer():
    """This walrus build allows only ONE sync wait per instruction; Tile
    emits several. Split extra waits into standalone EventSemaphore
    instructions on the same engine (same blocking semantics)."""
    import orjson
    import concourse.bass2jax as b2j
    import concourse.bass_utils as bu

    if getattr(b2j, "_legalizer_installed", False):
        return

    def legalize(bir):
        d = orjson.loads(bir)
        ctr = 0
        changed = False
        for fn in d.get("functions", []):
            for blk in fn.get("blocks", []):
                new = []
                for inst in blk.get("instructions", []):
                    si = inst.get("sync_info")
                    waits = si.get("on_wait", []) if si else []
                    if len(waits) > 1:
                        changed = True
                        for w in waits[:-1]:
                            ctr += 1
                            new.append({
                                "debug": inst.get("debug", 0),
                                "engine": inst["engine"],
                                "ins": [], "outs": [],
                                "name": f"lgw{ctr}_{inst.get('name', '')}"[:64],
                                "opcode": "EventSemaphore",
                                "sync_info": {"on_update": [], "on_wait": [w]},
                            })
                        si["on_wait"] = [waits[-1]]
                    new.append(inst)
                blk["instructions"] = new
        return orjson.dumps(d) if changed else bir

    orig = bu.compile_bir_kernel

    def wrapped(bir_json, tmpdir, neff_name="file.neff"):
        if isinstance(bir_json, str):
            bir_json = bir_json.encode()
        return orig(legalize(bir_json), tmpdir, neff_name=neff_name)

    b2j.compile_bir_kernel = wrapped
    b2j._legalizer_installed = True


# ------------------------------------------------------------------ host prep

def _host_prep(x, W, a, edge_index):
    import ml_dtypes

    x = np.asarray(x, np.float32)
    W_np = np.asarray(W, np.float32)
    a_np = np.asarray(a, np.float32)
    V, IN = x.shape
    row = np.asarray(edge_index[0]).astype(np.int64)
    col = np.asarray(edge_index[1]).astype(np.int64)

    ntiles = NCORES * TPC            # 392 table blocks == dest tiles
    nslots = ntiles * P              # 50176
    vpad = nslots
    PAD = vpad - V                   # trash rows 0..PAD-1; node n -> row n+PAD

    # destination scheduling: degree-sorted, tiles dealt round-robin
    deg = np.bincount(row, minlength=V)
    degp = np.concatenate([deg, np.zeros(nslots - V, np.int64)])
    order = np.argsort(-degp, kind="stable")
    tile_rows = order.reshape(ntiles, P)
    tile_maxdeg = np.where(tile_rows < V, deg[np.minimum(tile_rows, V - 1)], 0).max(1)
    gidx = np.arange(ntiles).reshape(TPC, NCORES)
    F_sched = np.maximum(tile_maxdeg[gidx].max(1), 1).astype(np.int64)
    nslots_e = int(F_sched.sum())

    # edges sorted by (row, col): per-row cols ascending
    eorder = np.lexsort((col, row))
    col_s = col[eorder]
    row_s = row[eorder]
    rstart = np.searchsorted(row_s, np.arange(V))
    rend = np.searchsorted(row_s, np.arange(V), side="right")

    wa1 = (W_np.astype(np.float64) @ a_np[:P].astype(np.float64)).astype(np.float32)
    wa2 = (W_np.astype(np.float64) @ a_np[P:].astype(np.float64)).astype(np.float32)

    # pair-interleaved xT: xT column k <-> table row
    #   r(k) = (k//256)*256 + 2*(k%128) + (k%256)//128,  node(r) = r - PAD
    k = np.arange(vpad)
    r_of_col = (k // 256) * 256 + 2 * (k % P) + (k % 256) // P
    node_of_col = r_of_col - PAD
    xT = np.zeros((IN, vpad), np.float32)
    real = node_of_col >= 0
    xT[:, real] = x.T[:, node_of_col[real]]
    trash_x = (-1e4 / float(wa2 @ wa2)) * wa2
    xT[:, ~real] = trash_x[:, None]

    rhs = np.zeros((IN, 130), np.float32)
    rhs[:, :P] = W_np
    rhs[:, P] = wa2
    rhs[:, P + 1] = wa1

    slot_off = np.concatenate([[0], np.cumsum(F_sched)])
    in_maps = []
    row_perm = np.empty((NCORES, TPC * P), np.int64)
    xT_bf = xT.astype(ml_dtypes.bfloat16)
    rhs_bf = rhs.astype(ml_dtypes.bfloat16)
    xTf = x.T
    needs_max = np.zeros(nslots_e, np.int64)   # per slot: max over cores

    for c in range(NCORES):
        offs = np.zeros((P, nslots_e), np.int32)   # trash row 0
        rows_of_core = np.empty(TPC * P, np.int64)
        for j in range(TPC):
            rl = tile_rows[j * NCORES + c]
            rows_of_core[j * P:(j + 1) * P] = rl
            o = slot_off[j]
            for p in range(P):
                r = rl[p]
                if r >= V:
                    continue
                n = rend[r] - rstart[r]
                offs[p, o:o + n] = col_s[rstart[r]:rstart[r] + n] + PAD
        row_perm[c] = rows_of_core
        needs_max = np.maximum(needs_max, offs.max(axis=0) + 1)
        xr = np.zeros((IN, TPC * P), np.float32)
        realr = rows_of_core < V
        xr[:, realr] = xTf[:, rows_of_core[realr]]
        in_maps.append({
            "xT": xT_bf, "rhs": rhs_bf,
            "xtr": np.ascontiguousarray(xr).astype(ml_dtypes.bfloat16),
            "offs": offs,
        })

    # per-slot table watermark in units of XB*P-row big-tile writes
    needs_bt = np.maximum(1, -(-needs_max // (XB * P))).astype(np.int64)

    meta = dict(F_sched=F_sched.tolist(), vt_tiles=ntiles,
                needs_bt=needs_bt.tolist(), row_perm=row_perm, V=V)
    return in_maps, meta


# ------------------------------------------------------------------ kernel build

def _build_kernel(F_sched, vt_tiles, needs_bt=None):
    import concourse.bass as bass
    import concourse.mybir as mybir
    import concourse.tile as tile

    F32 = mybir.dt.float32
    BF16 = mybir.dt.bfloat16
    I32 = mybir.dt.int32
    AF = mybir.ActivationFunctionType
    OP = mybir.AluOpType
    AX = mybir.AxisListType

    vpad = vt_tiles * P
    nrows = TPC * P
    Fmax = int(max(F_sched))
    nslots_e = int(sum(F_sched))
    nbt = vt_tiles // XB
    if needs_bt is None:
        needs_bt = [nbt] * nslots_e
    slot_off = [0]
    for f in F_sched:
        slot_off.append(slot_off[-1] + f)

    # emission units: (need, j, s0, s1); sorted by table watermark
    units = []
    nbands_of = {}
    for j in range(TPC):
        Fj = int(F_sched[j])
        # band starts [0, 2, 6, 10, ...]: a small first band lowers the
        # earliest table watermark, shortening the phase-1 lead-in
        starts = [0] + list(range(2, Fj, BAND)) if Fj > 2 else [0]
        nbands_of[j] = len(starts)
        for i, s0 in enumerate(starts):
            s1 = starts[i + 1] if i + 1 < len(starts) else Fj
            need = int(needs_bt[slot_off[j] + s1 - 1])
            units.append((need, j, s0, s1))
    units.sort(key=lambda u: (u[0], u[1], u[2]))

    nc = bass.Bass("TRN2")
    xT = nc.dram_tensor("xT", [256, vpad], BF16, kind="ExternalInput")
    rhs = nc.dram_tensor("rhs", [256, 130], BF16, kind="ExternalInput")
    xtr = nc.dram_tensor("xtr", [256, nrows], BF16, kind="ExternalInput")
    offs = nc.dram_tensor("offs", [P, nslots_e], I32, kind="ExternalInput")
    out = nc.dram_tensor("out", [nrows, P], F32, kind="ExternalOutput")

    with tile.TileContext(nc) as tc:
        with (
            tc.tile_pool(name="tab", bufs=1, space="DRAM") as tabpool,
            tc.tile_pool(name="const", bufs=1) as cpool,
            tc.tile_pool(name="xt", bufs=3) as xtpool,
            tc.tile_pool(name="tb", bufs=3) as tbpool,
            tc.tile_pool(name="meta", bufs=1) as mpool,
            tc.tile_pool(name="g", bufs=12) as gpool,
            tc.tile_pool(name="sm", bufs=2) as smpool,
            tc.tile_pool(name="pt", bufs=1) as ptpool,
            tc.tile_pool(name="ob", bufs=2) as opool,
            tc.tile_pool(name="ps", bufs=1, space="PSUM") as pspool,
            tc.tile_pool(name="pss", bufs=2, space="PSUM") as psspool,
        ):
            T_tile = tabpool.tile([vpad, TW], BF16)
            rhs0 = cpool.tile([P, 130], BF16)
            nc.sync.dma_start(rhs0[:], rhs[0:P, :])
            rhs1 = cpool.tile([P, 130], BF16)
            nc.sync.dma_start(rhs1[:], rhs[P:2 * P, :])

            # -------- phase-2 prep (runs under phase 1) --------
            offs_t = mpool.tile([P, nslots_e], I32)
            nc.sync.dma_start(offs_t[:], offs[:])
            xtr_t = mpool.tile([P, 2 * nrows], BF16)
            nc.sync.dma_start(xtr_t[:, 0:nrows], xtr[0:P, :])
            nc.sync.dma_start(xtr_t[:, nrows:2 * nrows], xtr[P:2 * P, :])
            sv_all = mpool.tile([P, TPC], F32)

            def emit_prep():
                for j in range(TPC):
                    ps_s = psspool.tile([P, 1], F32, tag="pss")
                    nc.tensor.matmul(ps_s[:], lhsT=xtr_t[:, j * P:(j + 1) * P],
                                     rhs=rhs0[:, 129:130], start=True, stop=False)
                    nc.tensor.matmul(
                        ps_s[:],
                        lhsT=xtr_t[:, nrows + j * P:nrows + (j + 1) * P],
                        rhs=rhs1[:, 129:130], start=False, stop=True)
                    nc.scalar.activation(sv_all[:, j:j + 1], ps_s[:], AF.Copy)

            # -------- phase 1: table build --------
            # the gather lead-in is paced by the first ~14 block writes, so
            # the s_src prep matmuls are deferred past them
            PREP_AT = min(14, nbt)
            for b in range(nbt):
                if b == PREP_AT:
                    emit_prep()
                xt0 = xtpool.tile([P, XB * P], BF16, tag="xt0")
                nc.sync.dma_start(xt0[:], xT[0:P, b * XB * P:(b + 1) * XB * P])
                xt1 = xtpool.tile([P, XB * P], BF16, tag="xt1")
                nc.sync.dma_start(xt1[:], xT[P:2 * P, b * XB * P:(b + 1) * XB * P])
                tb = tbpool.tile([P, XB * TW], BF16, tag="tb")
                tbf = tb[:].bitcast(F32)
                for q in range(XB):
                    ps = pspool.tile([P, 130], F32, tag=f"p{q % 6}")
                    nc.tensor.matmul(ps[:], lhsT=xt0[:, q * P:(q + 1) * P],
                                     rhs=rhs0[:], start=True, stop=False)
                    nc.tensor.matmul(ps[:], lhsT=xt1[:, q * P:(q + 1) * P],
                                     rhs=rhs1[:], start=False, stop=True)
                    feat_dst = tb[:, q * TW:q * TW + P]
                    if q % 2 == 0:
                        nc.vector.tensor_copy(feat_dst, ps[:, 0:P])
                        nc.scalar.activation(tbf[:, q * 66 + 64:q * 66 + 66],
                                             ps[:, P:P + 2], AF.Copy)
                    else:
                        nc.scalar.activation(feat_dst, ps[:, 0:P], AF.Copy)
                        nc.vector.tensor_copy(tbf[:, q * 66 + 64:q * 66 + 66],
                                              ps[:, P:P + 2])
                dst = bass.AP(T_tile.tensor, (b * XB * P) * TW,
                              [[2 * TW, P], [256 * TW, XB // 2], [1, 2 * TW]])
                nc.sync.dma_start(dst, tb[:])
            if nbt <= 14:
                emit_prep()

            # -------- phase 2: watermark-ordered gather + aggregate --------
            acc = {}
            phi = {}
            done_bands = {j: 0 for j in range(TPC)}
            outb = None
            for (need, j, s0, s1) in units:
                Fj = int(F_sched[j])
                o0 = slot_off[j]
                g = s1 - s0

                gt = gpool.tile([P, BAND * TW], BF16, tag="gt")
                for d in range(g):
                    nr = min(int(needs_bt[o0 + s0 + d]) * XB * P, vpad)
                    nc.gpsimd.indirect_dma_start(
                        out=gt[:, d * TW:(d + 1) * TW], out_offset=None,
                        in_=T_tile[0:nr, :],
                        in_offset=bass.IndirectOffsetOnAxis(
                            ap=offs_t[:, o0 + s0 + d:o0 + s0 + d + 1], axis=0),
                    )
                gtf = gt[:].bitcast(F32)

                if j not in phi:
                    phi_j = ptpool.tile([P, Fmax], F32, tag=f"phi{j}")
                    acc_j = ptpool.tile([P, P], F32, tag=f"acc{j}")
                    phi[j] = phi_j
                    acc[j] = acc_j
                u = smpool.tile([P, BAND], F32, tag="u")
                nc.vector.tensor_scalar(out=u[:, 0:g],
                                        in0=gtf[:, 64:64 + (g - 1) * 66 + 1:66],
                                        scalar1=sv_all[:, j:j + 1], scalar2=None,
                                        op0=OP.add)
                ua = smpool.tile([P, BAND], F32, tag="ua")
                nc.vector.tensor_scalar(out=ua[:, 0:g], in0=u[:, 0:g],
                                        scalar1=ALPHA, scalar2=None, op0=OP.mult)
                lr = smpool.tile([P, BAND], F32, tag="lr")
                nc.vector.tensor_tensor(out=lr[:, 0:g], in0=u[:, 0:g],
                                        in1=ua[:, 0:g], op=OP.max)
                nc.scalar.activation(phi[j][:, s0:s1], lr[:, 0:g], AF.Exp)

                for d in range(g):
                    sd = s0 + d
                    if sd == 0:
                        nc.vector.tensor_scalar(out=acc[j][:], in0=gt[:, 0:P],
                                                scalar1=phi[j][:, 0:1],
                                                scalar2=None, op0=OP.mult)
                    else:
                        nc.vector.scalar_tensor_tensor(
                            out=acc[j][:], in0=gt[:, d * TW:d * TW + P],
                            scalar=phi[j][:, sd:sd + 1], in1=acc[j][:],
                            op0=OP.mult, op1=OP.add)

                done_bands[j] += 1
                if done_bands[j] == nbands_of[j]:
                    den_raw = smpool.tile([P, 1], F32, tag="denr")
                    nc.vector.tensor_reduce(out=den_raw[:], in_=phi[j][:, 0:Fj],
                                            axis=AX.X, op=OP.add)
                    den = smpool.tile([P, 1], F32, tag="den")
                    nc.vector.tensor_scalar(out=den[:], in0=den_raw[:],
                                            scalar1=1e-30, scalar2=None, op0=OP.max)
                    rden = smpool.tile([P, 1], F32, tag="rden")
                    nc.vector.reciprocal(rden[:], den[:])
                    res = smpool.tile([P, P], F32, tag="res")
                    nc.vector.tensor_scalar(out=res[:], in0=acc[j][:],
                                            scalar1=rden[:], scalar2=None,
                                            op0=OP.mult)
                    t1 = smpool.tile([P, P], F32, tag="t1")
                    nc.vector.tensor_scalar(out=t1[:], in0=res[:], scalar1=0.0,
                                            scalar2=-1.0, op0=OP.max, op1=OP.add)
                    t2 = smpool.tile([P, P], F32, tag="t2")
                    nc.vector.tensor_scalar(out=t2[:], in0=res[:], scalar1=0.0,
                                            scalar2=None, op0=OP.min)
                    t3 = smpool.tile([P, P], F32, tag="t3")
                    nc.scalar.activation(t3[:], t2[:], AF.Exp)
                    outb = opool.tile([P, P], F32, tag="outb")
                    nc.vector.scalar_tensor_tensor(out=outb[:], in0=t3[:],
                                                   scalar=1.0, in1=t1[:],
                                                   op0=OP.mult, op1=OP.add)
                    dst = bass.AP(out, (j * P) * P, [[P, P], [1, P]])
                    nc.sync.dma_start(dst, outb[:])
    return nc


# ------------------------------------------------------------------ entry

def kernel(x, W, a, edge_index):
    _install_legalizer()
    from concourse.bass_utils import run_bass_kernel_spmd

    x = np.asarray(x)
    in_maps, meta = _host_prep(x, W, a, edge_index)
    nc = _build_kernel(meta["F_sched"], meta["vt_tiles"], meta["needs_bt"])
    res = run_bass_kernel_spmd(nc, in_maps, core_ids=list(range(NCORES)))

    V = meta["V"]
    row_perm = meta["row_perm"]
    full = np.zeros((V, P), np.float32)
    for c, r in enumerate(res.results):
        rp = row_perm[c]
        valid = rp < V
        full[rp[valid]] = r["out"][valid]
    return full



# revision 6
# speedup vs baseline: 1.5051x; 1.5051x over previous
"""Self-contained Trainium2 Bass kernel for the GAT layer problem
nn_GATLayer_57062935494774 (V=50000, E=800000, IN=256, OUT=128, alpha=0.2).

kernel(**inputs) takes the full unsharded inputs (x, W, a, edge_index),
distributes across 8 NeuronCores, and returns the full (V, 128) output.

v4 design (on top of the v3 watermark-overlap design):
  The per-slot indirect gather is a FIXED ~1.09us GpSimd instruction
  (measured invariant to payload bytes and near-invariant to descriptor
  count), so ~35% of the slots are moved off GpSimd entirely: for
  "recompute" tiles, the 128 per-edge rows of each slot are produced by
  a TensorE matmul whose lhsT is the host-pregathered xT columns of
  those edges (xe input) -- no gather, just 2 matmuls + 1 PSUM->SBUF
  copy per slot.  GpSimd keeps the remaining slots via SWDGE indirect
  gathers from a DRAM table, overlapped with the table build exactly as
  in v3 (per-prefix watermarks).
  The table + aggregation run in fp16: rows are 132 fp16 (264 B) =
  [128 feats | s_dst | pad], which kills the fp32-score bitcast
  machinery and halves DVE bytes (2x mode).  Scores stay fp32 on the
  score path (u/lrelu); phi and acc are fp16; denominators/ELU fp32.
"""

import numpy as np

P = 128
TW = 132          # table row width in fp16 elems (264 B): 129 used + 3 pad
ALPHA = 0.2
NCORES = 8
TPC = 49          # row tiles per core (8*49*128 = 50176 >= 50000)
XB = 8            # phase-1 node blocks per big tile (1024 rows per write)
BAND = 4          # slots per emission unit
REC_TARGET = 280  # slots per core moved to the TensorE recompute path


# ------------------------------------------------------------------ fixes

def _install_legalizer():
    """This walrus build allows only ONE sync wait per instruction; Tile
    emits several. Split extra waits into standalone EventSemaphore
    instructions on the same engine (same blocking semantics)."""
    import orjson
    import concourse.bass2jax as b2j
    import concourse.bass_utils as bu

    if getattr(b2j, "_legalizer_installed", False):
        return

    def legalize(bir):
        d = orjson.loads(bir)
        ctr = 0
        changed = False
        for fn in d.get("functions", []):
            for blk in fn.get("blocks", []):
                new = []
                for inst in blk.get("instructions", []):
                    si = inst.get("sync_info")
                    waits = si.get("on_wait", []) if si else []
                    if len(waits) > 1:
                        changed = True
                        for w in waits[:-1]:
                            ctr += 1
                            new.append({
                                "debug": inst.get("debug", 0),
                                "engine": inst["engine"],
                                "ins": [], "outs": [],
                                "name": f"lgw{ctr}_{inst.get('name', '')}"[:64],
                                "opcode": "EventSemaphore",
                                "sync_info": {"on_update": [], "on_wait": [w]},
                            })
                        si["on_wait"] = [waits[-1]]
                    new.append(inst)
                blk["instructions"] = new
        return orjson.dumps(d) if changed else bir

    orig = bu.compile_bir_kernel

    def wrapped(bir_json, tmpdir, neff_name="file.neff"):
        if isinstance(bir_json, str):
            bir_json = bir_json.encode()
        return orig(legalize(bir_json), tmpdir, neff_name=neff_name)

    b2j.compile_bir_kernel = wrapped
    b2j._legalizer_installed = True


# ------------------------------------------------------------------ host prep

def _host_prep(x, W, a, edge_index):
    import ml_dtypes

    x = np.asarray(x, np.float32)
    W_np = np.asarray(W, np.float32)
    a_np = np.asarray(a, np.float32)
    V, IN = x.shape
    row = np.asarray(edge_index[0]).astype(np.int64)
    col = np.asarray(edge_index[1]).astype(np.int64)

    ntiles = NCORES * TPC            # 392 table blocks == dest tiles
    nslots = ntiles * P              # 50176
    vpad = nslots
    PAD = vpad - V                   # trash rows 0..PAD-1; node n -> row n+PAD

    # destination scheduling: degree-sorted, tiles dealt round-robin
    deg = np.bincount(row, minlength=V)
    degp = np.concatenate([deg, np.zeros(nslots - V, np.int64)])
    order = np.argsort(-degp, kind="stable")
    tile_rows = order.reshape(ntiles, P)
    tile_maxdeg = np.where(tile_rows < V, deg[np.minimum(tile_rows, V - 1)], 0).max(1)
    gidx = np.arange(ntiles).reshape(TPC, NCORES)
    F_sched = np.maximum(tile_maxdeg[gidx].max(1), 1).astype(np.int64)
    nslots_e = int(F_sched.sum())

    # pick recompute tiles from the low-degree end until REC_TARGET slots
    rec_tiles = []
    acc = 0
    for j in range(TPC - 1, -1, -1):
        if acc >= REC_TARGET:
            break
        rec_tiles.append(j)
        acc += int(F_sched[j])
    rec_set = set(rec_tiles)

    # edges sorted by (row, col): per-row cols ascending
    eorder = np.lexsort((col, row))
    col_s = col[eorder]
    row_s = row[eorder]
    rstart = np.searchsorted(row_s, np.arange(V))
    rend = np.searchsorted(row_s, np.arange(V), side="right")

    wa1 = (W_np.astype(np.float64) @ a_np[:P].astype(np.float64)).astype(np.float32)
    wa2 = (W_np.astype(np.float64) @ a_np[P:].astype(np.float64)).astype(np.float32)

    # pair-interleaved xT: xT column k <-> table row
    #   r(k) = (k//256)*256 + 2*(k%128) + (k%256)//128,  node(r) = r - PAD
    k = np.arange(vpad)
    r_of_col = (k // 256) * 256 + 2 * (k % P) + (k % 256) // P
    node_of_col = r_of_col - PAD
    xT = np.zeros((IN, vpad), np.float32)
    real = node_of_col >= 0
    xT[:, real] = x.T[:, node_of_col[real]]
    trash_x = (-1e4 / float(wa2 @ wa2)) * wa2
    xT[:, ~real] = trash_x[:, None]

    rhs = np.zeros((IN, 130), np.float32)
    rhs[:, :P] = W_np
    rhs[:, P] = wa2
    rhs[:, P + 1] = wa1

    slot_off = np.concatenate([[0], np.cumsum(F_sched)])
    rec_slot_off = {}
    o = 0
    for j in sorted(rec_set):
        rec_slot_off[j] = o
        o += int(F_sched[j])
    nrec = o

    in_maps = []
    row_perm = np.empty((NCORES, TPC * P), np.int64)
    xT_bf = xT.astype(ml_dtypes.bfloat16)
    rhs_bf = rhs.astype(ml_dtypes.bfloat16)
    xTf = x.T
    needs_max = np.zeros(nslots_e, np.int64)   # per slot: max over cores

    for c in range(NCORES):
        offs = np.zeros((P, nslots_e), np.int32)   # trash row 0
        xe_src = np.full(max(nrec, 1) * P, -1, np.int64)
        rows_of_core = np.empty(TPC * P, np.int64)
        for j in range(TPC):
            rl = tile_rows[j * NCORES + c]
            rows_of_core[j * P:(j + 1) * P] = rl
            o = slot_off[j]
            is_rec = j in rec_set
            ro = rec_slot_off.get(j, 0)
            for p in range(P):
                r = rl[p]
                if r >= V:
                    continue
                n = rend[r] - rstart[r]
                cols = col_s[rstart[r]:rstart[r] + n]
                if is_rec:
                    xe_src[(ro + np.arange(n)) * P + p] = cols
                else:
                    offs[p, o:o + n] = cols + PAD
        xe = np.empty((IN, max(nrec, 1) * P), np.float32)
        xe[:] = trash_x[:, None]
        real_e = xe_src >= 0
        xe[:, real_e] = xTf[:, xe_src[real_e]]
        row_perm[c] = rows_of_core
        needs_max = np.maximum(needs_max, offs.max(axis=0) + 1)
        xr = np.zeros((IN, TPC * P), np.float32)
        realr = rows_of_core < V
        xr[:, realr] = xTf[:, rows_of_core[realr]]
        in_maps.append({
            "xT": xT_bf, "rhs": rhs_bf,
            "xtr": np.ascontiguousarray(xr).astype(ml_dtypes.bfloat16),
            "offs": offs,
            "xe": np.ascontiguousarray(xe).astype(ml_dtypes.bfloat16),
        })

    # per-slot table watermark in units of XB*P-row big-tile writes
    needs_bt = np.maximum(1, -(-needs_max // (XB * P))).astype(np.int64)

    meta = dict(F_sched=F_sched.tolist(), vt_tiles=ntiles,
                needs_bt=needs_bt.tolist(), row_perm=row_perm, V=V,
                rec_tiles=sorted(rec_set), rec_slot_off=rec_slot_off,
                nrec=nrec)
    return in_maps, meta


# ------------------------------------------------------------------ kernel build

def _build_kernel(meta):
    import concourse.bass as bass
    import concourse.mybir as mybir
    import concourse.tile as tile

    F_sched = meta["F_sched"]
    vt_tiles = meta["vt_tiles"]
    needs_bt = meta["needs_bt"]
    rec_set = set(meta["rec_tiles"])
    rec_slot_off = meta["rec_slot_off"]
    nrec = max(meta["nrec"], 1)

    F32 = mybir.dt.float32
    F16 = mybir.dt.float16
    BF16 = mybir.dt.bfloat16
    I32 = mybir.dt.int32
    AF = mybir.ActivationFunctionType
    OP = mybir.AluOpType
    AX = mybir.AxisListType

    vpad = vt_tiles * P
    nrows = TPC * P
    Fmax = int(max(F_sched))
    nslots_e = int(sum(F_sched))
    nbt = vt_tiles // XB
    slot_off = [0]
    for f in F_sched:
        slot_off.append(slot_off[-1] + f)

    # emission units: (kind, need, j, s0, s1)
    g_units = []
    r_units = []
    nbands_of = {}
    for j in range(TPC):
        Fj = int(F_sched[j])
        starts = [0] + list(range(2, Fj, BAND)) if Fj > 2 else [0]
        nbands_of[j] = len(starts)
        for i, s0 in enumerate(starts):
            s1 = starts[i + 1] if i + 1 < len(starts) else Fj
            if j in rec_set:
                r_units.append((j, s0, s1))
            else:
                need = int(needs_bt[slot_off[j] + s1 - 1])
                g_units.append((need, j, s0, s1))
    g_units.sort(key=lambda u: (u[0], u[1], u[2]))

    # merge the two streams by estimated readiness: gather unit i ready at
    # ~i * 1.3us/slot on GpSimd; recompute unit r ready only after phase 1
    # (~330us of TensorE) plus ~0.9us/slot of its own stream.
    merged = []
    gi, ri = 0, 0
    tg, tr = 0.0, 330.0
    while gi < len(g_units) or ri < len(r_units):
        if ri >= len(r_units):
            take_g = True
        elif gi >= len(g_units):
            take_g = False
        else:
            take_g = tg <= tr
        if take_g:
            u = g_units[gi]
            merged.append(("g",) + u)
            tg += 1.25 * (u[3] - u[2])
            gi += 1
        else:
            j, s0, s1 = r_units[ri]
            merged.append(("r", 0, j, s0, s1))
            tr += 0.95 * (s1 - s0)
            ri += 1

    nc = bass.Bass("TRN2")
    xT = nc.dram_tensor("xT", [256, vpad], BF16, kind="ExternalInput")
    rhs = nc.dram_tensor("rhs", [256, 130], BF16, kind="ExternalInput")
    xtr = nc.dram_tensor("xtr", [256, nrows], BF16, kind="ExternalInput")
    offs = nc.dram_tensor("offs", [P, nslots_e], I32, kind="ExternalInput")
    xe = nc.dram_tensor("xe", [256, nrec * P], BF16, kind="ExternalInput")
    out = nc.dram_tensor("out", [nrows, P], F32, kind="ExternalOutput")

    with tile.TileContext(nc) as tc:
        with (
            tc.tile_pool(name="tab", bufs=1, space="DRAM") as tabpool,
            tc.tile_pool(name="const", bufs=1) as cpool,
            tc.tile_pool(name="xt", bufs=3) as xtpool,
            tc.tile_pool(name="tb", bufs=3) as tbpool,
            tc.tile_pool(name="meta", bufs=1) as mpool,
            tc.tile_pool(name="g", bufs=12) as gpool,
            tc.tile_pool(name="xe", bufs=3) as xepool,
            tc.tile_pool(name="sm", bufs=2) as smpool,
            tc.tile_pool(name="pt", bufs=1) as ptpool,
            tc.tile_pool(name="ob", bufs=2) as opool,
            tc.tile_pool(name="ps", bufs=1, space="PSUM") as pspool,
            tc.tile_pool(name="pss", bufs=2, space="PSUM") as psspool,
        ):
            T_tile = tabpool.tile([vpad, TW], F16)
            rhs0 = cpool.tile([P, 130], BF16)
            nc.sync.dma_start(rhs0[:], rhs[0:P, :])
            rhs1 = cpool.tile([P, 130], BF16)
            nc.sync.dma_start(rhs1[:], rhs[P:2 * P, :])

            # -------- phase-2 prep (runs under phase 1) --------
            offs_t = mpool.tile([P, nslots_e], I32)
            nc.sync.dma_start(offs_t[:], offs[:])
            xtr_t = mpool.tile([P, 2 * nrows], BF16)
            nc.sync.dma_start(xtr_t[:, 0:nrows], xtr[0:P, :])
            nc.sync.dma_start(xtr_t[:, nrows:2 * nrows], xtr[P:2 * P, :])
            sv_all = mpool.tile([P, TPC], F32)

            def emit_prep():
                for j in range(TPC):
                    ps_s = psspool.tile([P, 1], F32, tag="pss")
                    nc.tensor.matmul(ps_s[:], lhsT=xtr_t[:, j * P:(j + 1) * P],
                                     rhs=rhs0[:, 129:130], start=True, stop=False)
                    nc.tensor.matmul(
                        ps_s[:],
                        lhsT=xtr_t[:, nrows + j * P:nrows + (j + 1) * P],
                        rhs=rhs1[:, 129:130], start=False, stop=True)
                    nc.scalar.activation(sv_all[:, j:j + 1], ps_s[:], AF.Copy)

            # -------- phase 1: table build --------
            PREP_AT = min(14, nbt)
            for b in range(nbt):
                if b == PREP_AT:
                    emit_prep()
                xt0 = xtpool.tile([P, XB * P], BF16, tag="xt0")
                nc.sync.dma_start(xt0[:], xT[0:P, b * XB * P:(b + 1) * XB * P])
                xt1 = xtpool.tile([P, XB * P], BF16, tag="xt1")
                nc.sync.dma_start(xt1[:], xT[P:2 * P, b * XB * P:(b + 1) * XB * P])
                tb = tbpool.tile([P, XB * TW], F16, tag="tb")
                for q in range(XB):
                    ps = pspool.tile([P, 130], F32, tag=f"p{q % 4}")
                    nc.tensor.matmul(ps[:], lhsT=xt0[:, q * P:(q + 1) * P],
                                     rhs=rhs0[:], start=True, stop=False)
                    nc.tensor.matmul(ps[:], lhsT=xt1[:, q * P:(q + 1) * P],
                                     rhs=rhs1[:], start=False, stop=True)
                    dst = tb[:, q * TW:q * TW + 129]
                    if q % 2 == 0:
                        nc.vector.tensor_copy(dst, ps[:, 0:129])
                    else:
                        nc.scalar.activation(dst, ps[:, 0:129], AF.Copy)
                dst = bass.AP(T_tile.tensor, (b * XB * P) * TW,
                              [[2 * TW, P], [256 * TW, XB // 2], [1, 2 * TW]])
                nc.sync.dma_start(dst, tb[:])
            if nbt <= 14:
                emit_prep()

            # -------- phase 2: merged gather + recompute stream --------
            acc = {}
            phi = {}
            done_bands = {j: 0 for j in range(TPC)}

            def band_ops(j, s0, s1, gt):
                Fj = int(F_sched[j])
                g = s1 - s0
                if j not in phi:
                    phi_j = ptpool.tile([P, Fmax], F32, tag=f"phi{j}", name=f"phi{j}")
                    acc_j = ptpool.tile([P, P], F16, tag=f"acc{j}", name=f"acc{j}")
                    phi[j] = phi_j
                    acc[j] = acc_j
                u = smpool.tile([P, BAND], F32, tag="u")
                nc.vector.tensor_scalar(
                    out=u[:, 0:g],
                    in0=gt[:, 128:128 + (g - 1) * TW + 1:TW],
                    scalar1=sv_all[:, j:j + 1], scalar2=None, op0=OP.add)
                ua = smpool.tile([P, BAND], F32, tag="ua")
                nc.vector.tensor_scalar(out=ua[:, 0:g], in0=u[:, 0:g],
                                        scalar1=ALPHA, scalar2=None, op0=OP.mult)
                lr = smpool.tile([P, BAND], F32, tag="lr")
                nc.vector.tensor_tensor(out=lr[:, 0:g], in0=u[:, 0:g],
                                        in1=ua[:, 0:g], op=OP.max)
                nc.scalar.activation(phi[j][:, s0:s1], lr[:, 0:g], AF.Exp)

                for d in range(g):
                    sd = s0 + d
                    if sd == 0:
                        nc.vector.tensor_scalar(out=acc[j][:],
                                                in0=gt[:, 0:P],
                                                scalar1=phi[j][:, 0:1],
                                                scalar2=None, op0=OP.mult)
                    else:
                        nc.vector.scalar_tensor_tensor(
                            out=acc[j][:], in0=gt[:, d * TW:d * TW + P],
                            scalar=phi[j][:, sd:sd + 1], in1=acc[j][:],
                            op0=OP.mult, op1=OP.add)

                done_bands[j] += 1
                if done_bands[j] == nbands_of[j]:
                    den_raw = smpool.tile([P, 1], F32, tag="denr")
                    nc.vector.tensor_reduce(out=den_raw[:], in_=phi[j][:, 0:Fj],
                                            axis=AX.X, op=OP.add)
                    den = smpool.tile([P, 1], F32, tag="den")
                    nc.vector.tensor_scalar(out=den[:], in0=den_raw[:],
                                            scalar1=1e-30, scalar2=None,
                                            op0=OP.max)
                    rden = smpool.tile([P, 1], F32, tag="rden")
                    nc.vector.reciprocal(rden[:], den[:])
                    res = smpool.tile([P, P], F32, tag="res")
                    nc.vector.tensor_scalar(out=res[:], in0=acc[j][:],
                                            scalar1=rden[:], scalar2=None,
                                            op0=OP.mult)
                    t1 = smpool.tile([P, P], F32, tag="t1")
                    nc.vector.tensor_scalar(out=t1[:], in0=res[:], scalar1=0.0,
                                            scalar2=-1.0, op0=OP.max, op1=OP.add)
                    t2 = smpool.tile([P, P], F32, tag="t2")
                    nc.vector.tensor_scalar(out=t2[:], in0=res[:], scalar1=0.0,
                                            scalar2=None, op0=OP.min)
                    t3 = smpool.tile([P, P], F32, tag="t3")
                    nc.scalar.activation(t3[:], t2[:], AF.Exp)
                    outb = opool.tile([P, P], F32, tag="outb")
                    nc.vector.scalar_tensor_tensor(out=outb[:], in0=t3[:],
                                                   scalar=1.0, in1=t1[:],
                                                   op0=OP.mult, op1=OP.add)
                    dst = bass.AP(out, (j * P) * P, [[P, P], [1, P]])
                    nc.sync.dma_start(dst, outb[:])

            for (kind, need, j, s0, s1) in merged:
                o0 = slot_off[j]
                g = s1 - s0
                gt = gpool.tile([P, BAND * TW], F16, tag="gt")
                if kind == "g":
                    for d in range(g):
                        nr = min(int(needs_bt[o0 + s0 + d]) * XB * P, vpad)
                        nc.gpsimd.indirect_dma_start(
                            out=gt[:, d * TW:(d + 1) * TW], out_offset=None,
                            in_=T_tile[0:nr, :],
                            in_offset=bass.IndirectOffsetOnAxis(
                                ap=offs_t[:, o0 + s0 + d:o0 + s0 + d + 1],
                                axis=0),
                        )
                else:
                    ro = rec_slot_off[j]
                    xe_t = xepool.tile([P, 2 * BAND * P], BF16, tag="xe")
                    nc.sync.dma_start(
                        xe_t[:, 0:g * P],
                        xe[0:P, (ro + s0) * P:(ro + s1) * P])
                    nc.sync.dma_start(
                        xe_t[:, BAND * P:BAND * P + g * P],
                        xe[P:2 * P, (ro + s0) * P:(ro + s1) * P])
                    for d in range(g):
                        ps = pspool.tile([P, 130], F32, tag=f"r{d % 2}")
                        nc.tensor.matmul(
                            ps[:], lhsT=xe_t[:, d * P:(d + 1) * P],
                            rhs=rhs0[:], start=True, stop=False)
                        nc.tensor.matmul(
                            ps[:], lhsT=xe_t[:, BAND * P + d * P:BAND * P + (d + 1) * P],
                            rhs=rhs1[:], start=False, stop=True)
                        nc.scalar.activation(gt[:, d * TW:d * TW + 129],
                                             ps[:, 0:129], AF.Copy)
                band_ops(j, s0, s1, gt)
    return nc


# ------------------------------------------------------------------ entry

def kernel(x, W, a, edge_index):
    _install_legalizer()
    from concourse.bass_utils import run_bass_kernel_spmd

    x = np.asarray(x)
    in_maps, meta = _host_prep(x, W, a, edge_index)
    nc = _build_kernel(meta)
    res = run_bass_kernel_spmd(nc, in_maps, core_ids=list(range(NCORES)))

    V = meta["V"]
    row_perm = meta["row_perm"]
    full = np.zeros((V, P), np.float32)
    for c, r in enumerate(res.results):
        rp = row_perm[c]
        valid = rp < V
        full[rp[valid]] = r["out"][valid]
    return full


# revision 8
# speedup vs baseline: 1.7386x; 1.1552x over previous
"""Self-contained Trainium2 Bass kernel for the GAT layer problem
nn_GATLayer_57062935494774 (V=50000, E=800000, IN=256, OUT=128, alpha=0.2).

kernel(**inputs) takes the full unsharded inputs (x, W, a, edge_index),
distributes across 8 NeuronCores, and returns the full (V, 128) output.

v4 design (on top of the v3 watermark-overlap design):
  The per-slot indirect gather is a FIXED ~1.09us GpSimd instruction
  (measured invariant to payload bytes and near-invariant to descriptor
  count), so ~35% of the slots are moved off GpSimd entirely: for
  "recompute" tiles, the 128 per-edge rows of each slot are produced by
  a TensorE matmul whose lhsT is the host-pregathered xT columns of
  those edges (xe input) -- no gather, just 2 matmuls + 1 PSUM->SBUF
  copy per slot.  GpSimd keeps the remaining slots via SWDGE indirect
  gathers from a DRAM table, overlapped with the table build exactly as
  in v3 (per-prefix watermarks).
  The table + aggregation run in fp16: rows are 132 fp16 (264 B) =
  [128 feats | s_dst | pad], which kills the fp32-score bitcast
  machinery and halves DVE bytes (2x mode).  Scores stay fp32 on the
  score path (u/lrelu); phi and acc are fp16; denominators/ELU fp32.
"""

import numpy as np

P = 128
TW = 132          # table row width in fp16 elems (264 B): 129 used + 3 pad
ALPHA = 0.2
NCORES = 8
TPC = 49          # row tiles per core (8*49*128 = 50176 >= 50000)
XB = 8            # phase-1 node blocks per big tile (1024 rows per write)
BAND = 4          # slots per emission unit
REC_TARGET = 500  # slots per core moved to the TensorE recompute path


# ------------------------------------------------------------------ fixes

def _install_legalizer():
    """This walrus build allows only ONE sync wait per instruction; Tile
    emits several. Split extra waits into standalone EventSemaphore
    instructions on the same engine (same blocking semantics)."""
    import orjson
    import concourse.bass2jax as b2j
    import concourse.bass_utils as bu

    if getattr(b2j, "_legalizer_installed", False):
        return

    def legalize(bir):
        d = orjson.loads(bir)
        ctr = 0
        changed = False
        for fn in d.get("functions", []):
            for blk in fn.get("blocks", []):
                new = []
                for inst in blk.get("instructions", []):
                    si = inst.get("sync_info")
                    waits = si.get("on_wait", []) if si else []
                    if len(waits) > 1:
                        changed = True
                        for w in waits[:-1]:
                            ctr += 1
                            new.append({
                                "debug": inst.get("debug", 0),
                                "engine": inst["engine"],
                                "ins": [], "outs": [],
                                "name": f"lgw{ctr}_{inst.get('name', '')}"[:64],
                                "opcode": "EventSemaphore",
                                "sync_info": {"on_update": [], "on_wait": [w]},
                            })
                        si["on_wait"] = [waits[-1]]
                    new.append(inst)
                blk["instructions"] = new
        return orjson.dumps(d) if changed else bir

    orig = bu.compile_bir_kernel

    def wrapped(bir_json, tmpdir, neff_name="file.neff"):
        if isinstance(bir_json, str):
            bir_json = bir_json.encode()
        return orig(legalize(bir_json), tmpdir, neff_name=neff_name)

    b2j.compile_bir_kernel = wrapped
    b2j._legalizer_installed = True


# ------------------------------------------------------------------ host prep

def _host_prep(x, W, a, edge_index):
    import ml_dtypes

    x = np.asarray(x, np.float32)
    W_np = np.asarray(W, np.float32)
    a_np = np.asarray(a, np.float32)
    V, IN = x.shape
    row = np.asarray(edge_index[0]).astype(np.int64)
    col = np.asarray(edge_index[1]).astype(np.int64)

    ntiles = NCORES * TPC            # 392 table blocks == dest tiles
    nslots = ntiles * P              # 50176
    vpad = nslots
    PAD = vpad - V                   # trash rows 0..PAD-1; node n -> row n+PAD

    # destination scheduling: degree-sorted, tiles dealt round-robin
    deg = np.bincount(row, minlength=V)
    degp = np.concatenate([deg, np.zeros(nslots - V, np.int64)])
    order = np.argsort(-degp, kind="stable")
    tile_rows = order.reshape(ntiles, P)
    tile_maxdeg = np.where(tile_rows < V, deg[np.minimum(tile_rows, V - 1)], 0).max(1)
    gidx = np.arange(ntiles).reshape(TPC, NCORES)
    F_sched = np.maximum(tile_maxdeg[gidx].max(1), 1).astype(np.int64)
    nslots_e = int(F_sched.sum())

    # pick recompute tiles from the low-degree end until REC_TARGET slots
    rec_tiles = []
    acc = 0
    for j in range(TPC - 1, -1, -1):
        if acc >= REC_TARGET:
            break
        rec_tiles.append(j)
        acc += int(F_sched[j])
    rec_set = set(rec_tiles)

    # edges sorted by (row, col): per-row cols ascending
    eorder = np.lexsort((col, row))
    col_s = col[eorder]
    row_s = row[eorder]
    rstart = np.searchsorted(row_s, np.arange(V))
    rend = np.searchsorted(row_s, np.arange(V), side="right")

    wa1 = (W_np.astype(np.float64) @ a_np[:P].astype(np.float64)).astype(np.float32)
    wa2 = (W_np.astype(np.float64) @ a_np[P:].astype(np.float64)).astype(np.float32)

    # pair-interleaved xT: xT column k <-> table row
    #   r(k) = (k//256)*256 + 2*(k%128) + (k%256)//128,  node(r) = r - PAD
    k = np.arange(vpad)
    r_of_col = (k // 256) * 256 + 2 * (k % P) + (k % 256) // P
    node_of_col = r_of_col - PAD
    xT = np.zeros((IN, vpad), np.float32)
    real = node_of_col >= 0
    xT[:, real] = x.T[:, node_of_col[real]]
    trash_x = (-1e4 / float(wa2 @ wa2)) * wa2
    xT[:, ~real] = trash_x[:, None]

    rhs = np.zeros((IN, 130), np.float32)
    rhs[:, :P] = W_np
    rhs[:, P] = wa2
    rhs[:, P + 1] = wa1

    slot_off = np.concatenate([[0], np.cumsum(F_sched)])
    rec_slot_off = {}
    o = 0
    for j in sorted(rec_set):
        rec_slot_off[j] = o
        o += int(F_sched[j])
    nrec = o

    in_maps = []
    row_perm = np.empty((NCORES, TPC * P), np.int64)
    xT_bf = xT.astype(ml_dtypes.bfloat16)
    rhs_bf = rhs.astype(ml_dtypes.bfloat16)
    xTf = x.T
    needs_max = np.zeros(nslots_e, np.int64)   # per slot: max over cores

    for c in range(NCORES):
        offs = np.zeros((P, nslots_e), np.int32)   # trash row 0
        xe_src = np.full(max(nrec, 1) * P, -1, np.int64)
        rows_of_core = np.empty(TPC * P, np.int64)
        for j in range(TPC):
            rl = tile_rows[j * NCORES + c]
            rows_of_core[j * P:(j + 1) * P] = rl
            o = slot_off[j]
            is_rec = j in rec_set
            ro = rec_slot_off.get(j, 0)
            for p in range(P):
                r = rl[p]
                if r >= V:
                    continue
                n = rend[r] - rstart[r]
                cols = col_s[rstart[r]:rstart[r] + n]
                if is_rec:
                    xe_src[(ro + np.arange(n)) * P + p] = cols
                else:
                    offs[p, o:o + n] = cols + PAD
        xe = np.empty((IN, max(nrec, 1) * P), np.float32)
        xe[:] = trash_x[:, None]
        real_e = xe_src >= 0
        xe[:, real_e] = xTf[:, xe_src[real_e]]
        row_perm[c] = rows_of_core
        needs_max = np.maximum(needs_max, offs.max(axis=0) + 1)
        xr = np.zeros((IN, TPC * P), np.float32)
        realr = rows_of_core < V
        xr[:, realr] = xTf[:, rows_of_core[realr]]
        in_maps.append({
            "xT": xT_bf, "rhs": rhs_bf,
            "xtr": np.ascontiguousarray(xr).astype(ml_dtypes.bfloat16),
            "offs": offs,
            "xe": np.ascontiguousarray(xe).astype(ml_dtypes.bfloat16),
        })

    # per-slot table watermark in units of XB*P-row big-tile writes
    needs_bt = np.maximum(1, -(-needs_max // (XB * P))).astype(np.int64)

    meta = dict(F_sched=F_sched.tolist(), vt_tiles=ntiles,
                needs_bt=needs_bt.tolist(), row_perm=row_perm, V=V,
                rec_tiles=sorted(rec_set), rec_slot_off=rec_slot_off,
                nrec=nrec)
    return in_maps, meta


# ------------------------------------------------------------------ kernel build

def _build_kernel(meta):
    import concourse.bass as bass
    import concourse.mybir as mybir
    import concourse.tile as tile

    F_sched = meta["F_sched"]
    vt_tiles = meta["vt_tiles"]
    needs_bt = meta["needs_bt"]
    rec_set = set(meta["rec_tiles"])
    rec_slot_off = meta["rec_slot_off"]
    nrec = max(meta["nrec"], 1)

    F32 = mybir.dt.float32
    F16 = mybir.dt.float16
    BF16 = mybir.dt.bfloat16
    I32 = mybir.dt.int32
    AF = mybir.ActivationFunctionType
    OP = mybir.AluOpType
    AX = mybir.AxisListType

    vpad = vt_tiles * P
    nrows = TPC * P
    Fmax = int(max(F_sched))
    nslots_e = int(sum(F_sched))
    nbt = vt_tiles // XB
    slot_off = [0]
    for f in F_sched:
        slot_off.append(slot_off[-1] + f)

    # emission units: (kind, need, j, s0, s1)
    g_units = []
    r_units = []
    nbands_of = {}
    for j in range(TPC):
        Fj = int(F_sched[j])
        starts = [0] + list(range(2, Fj, BAND)) if Fj > 2 else [0]
        nbands_of[j] = len(starts)
        for i, s0 in enumerate(starts):
            s1 = starts[i + 1] if i + 1 < len(starts) else Fj
            if j in rec_set:
                r_units.append((j, s0, s1))
            else:
                need = int(needs_bt[slot_off[j] + s1 - 1])
                g_units.append((need, j, s0, s1))
    g_units.sort(key=lambda u: (u[0], u[1], u[2]))

    # merge the two streams by estimated readiness: gather unit i ready at
    # ~i * 1.3us/slot on GpSimd; recompute unit r ready only after phase 1
    # (~330us of TensorE) plus ~0.9us/slot of its own stream.
    merged = []
    gi, ri = 0, 0
    tg, tr = 0.0, 160.0
    while gi < len(g_units) or ri < len(r_units):
        if ri >= len(r_units):
            take_g = True
        elif gi >= len(g_units):
            take_g = False
        else:
            take_g = tg <= tr
        if take_g:
            u = g_units[gi]
            merged.append(("g",) + u)
            tg += 1.25 * (u[3] - u[2])
            gi += 1
        else:
            j, s0, s1 = r_units[ri]
            merged.append(("r", 0, j, s0, s1))
            tr += 0.6 * (s1 - s0)
            ri += 1

    nc = bass.Bass("TRN2")
    xT = nc.dram_tensor("xT", [256, vpad], BF16, kind="ExternalInput")
    rhs = nc.dram_tensor("rhs", [256, 130], BF16, kind="ExternalInput")
    xtr = nc.dram_tensor("xtr", [256, nrows], BF16, kind="ExternalInput")
    offs = nc.dram_tensor("offs", [P, nslots_e], I32, kind="ExternalInput")
    xe = nc.dram_tensor("xe", [256, nrec * P], BF16, kind="ExternalInput")
    out = nc.dram_tensor("out", [nrows, P], F32, kind="ExternalOutput")

    with tile.TileContext(nc) as tc:
        with (
            tc.tile_pool(name="tab", bufs=1, space="DRAM") as tabpool,
            tc.tile_pool(name="const", bufs=1) as cpool,
            tc.tile_pool(name="xt", bufs=3) as xtpool,
            tc.tile_pool(name="tb", bufs=3) as tbpool,
            tc.tile_pool(name="meta", bufs=1) as mpool,
            tc.tile_pool(name="g", bufs=12) as gpool,
            tc.tile_pool(name="xe", bufs=3) as xepool,
            tc.tile_pool(name="sm", bufs=2) as smpool,
            tc.tile_pool(name="pt", bufs=1) as ptpool,
            tc.tile_pool(name="ob", bufs=2) as opool,
            tc.tile_pool(name="ps", bufs=1, space="PSUM") as pspool,
            tc.tile_pool(name="pss", bufs=2, space="PSUM") as psspool,
        ):
            T_tile = tabpool.tile([vpad, TW], F16)
            rhs0 = cpool.tile([P, 130], BF16)
            nc.sync.dma_start(rhs0[:], rhs[0:P, :])
            rhs1 = cpool.tile([P, 130], BF16)
            nc.sync.dma_start(rhs1[:], rhs[P:2 * P, :])

            # -------- phase-2 prep (runs under phase 1) --------
            offs_t = mpool.tile([P, nslots_e], I32)
            nc.sync.dma_start(offs_t[:], offs[:])
            xtr_t = mpool.tile([P, 2 * nrows], BF16)
            nc.sync.dma_start(xtr_t[:, 0:nrows], xtr[0:P, :])
            nc.sync.dma_start(xtr_t[:, nrows:2 * nrows], xtr[P:2 * P, :])
            sv_all = mpool.tile([P, TPC], F32)

            def emit_prep():
                for j in range(TPC):
                    ps_s = psspool.tile([P, 1], F32, tag="pss")
                    nc.tensor.matmul(ps_s[:], lhsT=xtr_t[:, j * P:(j + 1) * P],
                                     rhs=rhs0[:, 129:130], start=True, stop=False)
                    nc.tensor.matmul(
                        ps_s[:],
                        lhsT=xtr_t[:, nrows + j * P:nrows + (j + 1) * P],
                        rhs=rhs1[:, 129:130], start=False, stop=True)
                    nc.scalar.activation(sv_all[:, j:j + 1], ps_s[:], AF.Copy)

            # -------- phase 1: table build --------
            PREP_AT = min(14, nbt)
            for b in range(nbt):
                if b == PREP_AT:
                    emit_prep()
                xt0 = xtpool.tile([P, XB * P], BF16, tag="xt0")
                nc.sync.dma_start(xt0[:], xT[0:P, b * XB * P:(b + 1) * XB * P])
                xt1 = xtpool.tile([P, XB * P], BF16, tag="xt1")
                nc.sync.dma_start(xt1[:], xT[P:2 * P, b * XB * P:(b + 1) * XB * P])
                tb = tbpool.tile([P, XB * TW], F16, tag="tb")
                for q in range(XB):
                    ps = pspool.tile([P, 130], F32, tag=f"p{q % 4}")
                    nc.tensor.matmul(ps[:], lhsT=xt0[:, q * P:(q + 1) * P],
                                     rhs=rhs0[:], start=True, stop=False)
                    nc.tensor.matmul(ps[:], lhsT=xt1[:, q * P:(q + 1) * P],
                                     rhs=rhs1[:], start=False, stop=True)
                    dst = tb[:, q * TW:q * TW + 129]
                    if q % 2 == 0:
                        nc.vector.tensor_copy(dst, ps[:, 0:129])
                    else:
                        nc.scalar.activation(dst, ps[:, 0:129], AF.Copy)
                dst = bass.AP(T_tile.tensor, (b * XB * P) * TW,
                              [[2 * TW, P], [256 * TW, XB // 2], [1, 2 * TW]])
                nc.sync.dma_start(dst, tb[:])
            if nbt <= 14:
                emit_prep()

            # -------- phase 2: merged gather + recompute stream --------
            acc = {}
            phi = {}
            done_bands = {j: 0 for j in range(TPC)}

            def band_ops(j, s0, s1, gt):
                Fj = int(F_sched[j])
                g = s1 - s0
                if j not in phi:
                    phi_j = ptpool.tile([P, Fmax], F32, tag=f"phi{j}", name=f"phi{j}")
                    acc_j = ptpool.tile([P, P], F16, tag=f"acc{j}", name=f"acc{j}")
                    phi[j] = phi_j
                    acc[j] = acc_j
                u = smpool.tile([P, BAND], F32, tag="u")
                nc.vector.tensor_scalar(
                    out=u[:, 0:g],
                    in0=gt[:, 128:128 + (g - 1) * TW + 1:TW],
                    scalar1=sv_all[:, j:j + 1], scalar2=None, op0=OP.add)
                ua = smpool.tile([P, BAND], F32, tag="ua")
                nc.vector.tensor_scalar(out=ua[:, 0:g], in0=u[:, 0:g],
                                        scalar1=ALPHA, scalar2=None, op0=OP.mult)
                lr = smpool.tile([P, BAND], F32, tag="lr")
                nc.vector.tensor_tensor(out=lr[:, 0:g], in0=u[:, 0:g],
                                        in1=ua[:, 0:g], op=OP.max)
                nc.scalar.activation(phi[j][:, s0:s1], lr[:, 0:g], AF.Exp)

                for d in range(g):
                    sd = s0 + d
                    if sd == 0:
                        nc.vector.tensor_scalar(out=acc[j][:],
                                                in0=gt[:, 0:P],
                                                scalar1=phi[j][:, 0:1],
                                                scalar2=None, op0=OP.mult)
                    else:
                        nc.vector.scalar_tensor_tensor(
                            out=acc[j][:], in0=gt[:, d * TW:d * TW + P],
                            scalar=phi[j][:, sd:sd + 1], in1=acc[j][:],
                            op0=OP.mult, op1=OP.add)

                done_bands[j] += 1
                if done_bands[j] == nbands_of[j]:
                    den_raw = smpool.tile([P, 1], F32, tag="denr")
                    nc.vector.tensor_reduce(out=den_raw[:], in_=phi[j][:, 0:Fj],
                                            axis=AX.X, op=OP.add)
                    den = smpool.tile([P, 1], F32, tag="den")
                    nc.vector.tensor_scalar(out=den[:], in0=den_raw[:],
                                            scalar1=1e-30, scalar2=None,
                                            op0=OP.max)
                    rden = smpool.tile([P, 1], F32, tag="rden")
                    nc.vector.reciprocal(rden[:], den[:])
                    res = smpool.tile([P, P], F32, tag="res")
                    nc.vector.tensor_scalar(out=res[:], in0=acc[j][:],
                                            scalar1=rden[:], scalar2=None,
                                            op0=OP.mult)
                    t1 = smpool.tile([P, P], F32, tag="t1")
                    nc.vector.tensor_scalar(out=t1[:], in0=res[:], scalar1=0.0,
                                            scalar2=-1.0, op0=OP.max, op1=OP.add)
                    t2 = smpool.tile([P, P], F32, tag="t2")
                    nc.vector.tensor_scalar(out=t2[:], in0=res[:], scalar1=0.0,
                                            scalar2=None, op0=OP.min)
                    t3 = smpool.tile([P, P], F32, tag="t3")
                    nc.scalar.activation(t3[:], t2[:], AF.Exp)
                    outb = opool.tile([P, P], F32, tag="outb")
                    nc.vector.scalar_tensor_tensor(out=outb[:], in0=t3[:],
                                                   scalar=1.0, in1=t1[:],
                                                   op0=OP.mult, op1=OP.add)
                    dst = bass.AP(out, (j * P) * P, [[P, P], [1, P]])
                    nc.sync.dma_start(dst, outb[:])

            for (kind, need, j, s0, s1) in merged:
                o0 = slot_off[j]
                g = s1 - s0
                gt = gpool.tile([P, BAND * TW], F16, tag="gt")
                if kind == "g":
                    for d in range(g):
                        nr = min(int(needs_bt[o0 + s0 + d]) * XB * P, vpad)
                        nc.gpsimd.indirect_dma_start(
                            out=gt[:, d * TW:(d + 1) * TW], out_offset=None,
                            in_=T_tile[0:nr, :],
                            in_offset=bass.IndirectOffsetOnAxis(
                                ap=offs_t[:, o0 + s0 + d:o0 + s0 + d + 1],
                                axis=0),
                        )
                else:
                    ro = rec_slot_off[j]
                    xe_t = xepool.tile([P, 2 * BAND * P], BF16, tag="xe")
                    nc.sync.dma_start(
                        xe_t[:, 0:g * P],
                        xe[0:P, (ro + s0) * P:(ro + s1) * P])
                    nc.sync.dma_start(
                        xe_t[:, BAND * P:BAND * P + g * P],
                        xe[P:2 * P, (ro + s0) * P:(ro + s1) * P])
                    for d in range(g):
                        ps = pspool.tile([P, 130], F32, tag=f"r{d % 2}")
                        nc.tensor.matmul(
                            ps[:], lhsT=xe_t[:, d * P:(d + 1) * P],
                            rhs=rhs0[:], start=True, stop=False)
                        nc.tensor.matmul(
                            ps[:], lhsT=xe_t[:, BAND * P + d * P:BAND * P + (d + 1) * P],
                            rhs=rhs1[:], start=False, stop=True)
                        if d % 2 == 0:
                            nc.scalar.activation(gt[:, d * TW:d * TW + 129],
                                                 ps[:, 0:129], AF.Copy)
                        else:
                            nc.vector.tensor_copy(gt[:, d * TW:d * TW + 129],
                                                  ps[:, 0:129])
                band_ops(j, s0, s1, gt)
    return nc


# ------------------------------------------------------------------ entry

def kernel(x, W, a, edge_index):
    _install_legalizer()
    from concourse.bass_utils import run_bass_kernel_spmd

    x = np.asarray(x)
    in_maps, meta = _host_prep(x, W, a, edge_index)
    nc = _build_kernel(meta)
    res = run_bass_kernel_spmd(nc, in_maps, core_ids=list(range(NCORES)))

    V = meta["V"]
    row_perm = meta["row_perm"]
    full = np.zeros((V, P), np.float32)
    for c, r in enumerate(res.results):
        rp = row_perm[c]
        valid = rp < V
        full[rp[valid]] = r["out"][valid]
    return full
